# revision 1
# baseline (speedup 1.0000x reference)
"""Self-contained Trainium2 Bass kernel: 16-head attention with RoPE (B=2, S=2048, D=2048).

Sharding: 8 cores = 2 (batch) x 4 (head groups of 4 heads / 512 cols).
Per core: QKV projections for its head group -> RoPE -> causal attention ->
AllGather of attention outputs (X^T) within the 4-core batch group ->
column-sharded output projection. Host assembles by concatenation only.

The whole kernel is one software pipeline over 4 q-chunks of 512:
  chunk c: V(s-tiles 4c..4c+3), K(chunk c)+RoPE, Q(chunk c)+RoPE,
           attention(c) [causal: k-tiles 0..4c+3], AllGather(c),
           outproj(c-1)
so the AllGathers overlap projection/attention compute of later chunks.

Dataflow is fully "transposed" so no on-chip transposes are needed:
  hiddenT [d, s] (host-pretransposed, bf16), streamed per chunk
  QT/KT   [dh, s] per head  (projection emits head-dim-major directly)
  S^T     [k, q] scores     (lhsT = KT tile, rhs = QT)
  P^T     [k, q] = exp(S^T + mask^T)   (no max subtraction; scores are O(1))
  colsums via all-ones [128,128] matmul -> sums arrive partition-broadcast
  O^T     [dh, q] = V^T @ P^T          (lhsT = V natural [s, dh])
  X^T     AllGather on first axis, chunked along q
  out     [s, oc] (lhsT = X^T block, rhs = WoT)
RoPE de-interleave is folded into a host-side row permutation of Wq/Wk.
1/sqrt(DH) is folded into the Q rope tables.
"""

import math
from contextlib import ExitStack

import numpy as np
import ml_dtypes

B, S, D, H, DH = 2, 2048, 2048, 16, 128
NCORES = 8
GPC = 4            # cores per tensor-parallel group
HPC = H // GPC     # heads per core (4)
CW = HPC * DH      # 512 columns per core
NEG = -1e9
BF = ml_dtypes.bfloat16
QCH = 512          # q-chunk (moving free dim)
NQC = S // QCH     # 4
NDT = D // 128     # 16 d-tiles
NST = S // 128     # 16 s-tiles

REPLICA_GROUPS = [[0, 1, 2, 3], [4, 5, 6, 7]]

_built = {}


def _build(causal: bool, use_bias: bool):
    import concourse.bass as bass
    import concourse.tile as tile
    from concourse import bacc, mybir
    from concourse.tile_rust import add_dep_helper

    f32, bf16 = mybir.dt.float32, mybir.dt.bfloat16
    EXP = mybir.ActivationFunctionType.Exp
    IDN = mybir.ActivationFunctionType.Identity

    nc = bacc.Bacc("TRN2", target_bir_lowering=False, debug=False,
                   num_devices=NCORES)

    hT_d = nc.dram_tensor("hiddenT", [D, S], bf16, kind="ExternalInput")
    wq_d = nc.dram_tensor("wqT", [D, CW], bf16, kind="ExternalInput")
    wk_d = nc.dram_tensor("wkT", [D, CW], bf16, kind="ExternalInput")
    wv_d = nc.dram_tensor("wvT", [D, CW], bf16, kind="ExternalInput")
    wo_d = nc.dram_tensor("woT", [D, CW], bf16, kind="ExternalInput")
    cq_d = nc.dram_tensor("cq", [128, S], bf16, kind="ExternalInput")
    sq_d = nc.dram_tensor("sq", [128, S], bf16, kind="ExternalInput")
    ck_d = nc.dram_tensor("ck", [128, S], bf16, kind="ExternalInput")
    sk_d = nc.dram_tensor("sk", [128, S], bf16, kind="ExternalInput")
    if use_bias:
        bq_d = nc.dram_tensor("bqp", [128, HPC], f32, kind="ExternalInput")
        bk_d = nc.dram_tensor("bkp", [128, HPC], f32, kind="ExternalInput")
        bv_d = nc.dram_tensor("bv2", [1, CW], f32, kind="ExternalInput")
        bo_d = nc.dram_tensor("bo2", [1, CW], f32, kind="ExternalInput")
    if causal:
        dm_d = nc.dram_tensor("dmask", [128, 128], bf16, kind="ExternalInput")
    else:
        mT_d = nc.dram_tensor("maskT", [S, S], bf16, kind="ExternalInput")
    out_d = nc.dram_tensor("out", [S, CW], f32, kind="ExternalOutput")

    with tile.TileContext(nc) as tc, ExitStack() as ctx:
        hp = ctx.enter_context(tc.tile_pool(name="hp",
                                    bufs=2 * NDT + (8 if causal else 2)))
        xp = ctx.enter_context(tc.tile_pool(name="xp", bufs=NDT))
        wp = ctx.enter_context(tc.tile_pool(name="wp", bufs=4 * NDT))
        qkp = ctx.enter_context(tc.tile_pool(name="qkp", bufs=2 * HPC))
        vp = ctx.enter_context(tc.tile_pool(name="vp", bufs=NST))
        cst = ctx.enter_context(tc.tile_pool(name="cst", bufs=1))
        ptp = ctx.enter_context(tc.tile_pool(name="ptp", bufs=3))
        rp = ctx.enter_context(tc.tile_pool(name="rp", bufs=2))
        op = ctx.enter_context(tc.tile_pool(name="op", bufs=3))
        ps_mm = ctx.enter_context(tc.tile_pool(name="ps_mm", bufs=2, space="PSUM"))
        ps_mo = ctx.enter_context(tc.tile_pool(name="ps_mo", bufs=2, space="PSUM"))
        ps_s = ctx.enter_context(tc.tile_pool(name="ps_s", bufs=2, space="PSUM"))
        ps_pv = ctx.enter_context(tc.tile_pool(name="ps_pv", bufs=2, space="PSUM"))
        dram = ctx.enter_context(tc.tile_pool(name="dram", bufs=1, space="DRAM"))

        # ---- first-needed data first: hT(chunk 0) + Wv interleaved ----
        wv_sb, wq_sb, wk_sb, wo_sb = [], [], [], []
        hTc0 = []
        for dt in range(NDT):
            w = wp.tile([128, CW], bf16, tag="w", name=f"wv{dt}")
            nc.sync.dma_start(w[:], wv_d[dt * 128:(dt + 1) * 128, :])
            wv_sb.append(w)
            t = hp.tile([128, QCH], bf16, tag="hT", name=f"hT0_{dt}")
            nc.sync.dma_start(t[:], hT_d[dt * 128:(dt + 1) * 128, 0:QCH])
            hTc0.append(t)
        for dt in range(NDT):
            w = wp.tile([128, CW], bf16, tag="w", name=f"wk{dt}")
            nc.sync.dma_start(w[:], wk_d[dt * 128:(dt + 1) * 128, :])
            wk_sb.append(w)

        # ---- constants ----
        cq_sb = cst.tile([128, S], bf16, tag="cq", name="cq_sb")
        sq_sb = cst.tile([128, S], bf16, tag="sq", name="sq_sb")
        ck_sb = cst.tile([128, S], bf16, tag="ck", name="ck_sb")
        sk_sb = cst.tile([128, S], bf16, tag="sk", name="sk_sb")
        nc.sync.dma_start(ck_sb[:], ck_d[:])
        nc.sync.dma_start(sk_sb[:], sk_d[:])
        for dt in range(NDT):
            w = wp.tile([128, CW], bf16, tag="w", name=f"wq{dt}")
            nc.sync.dma_start(w[:], wq_d[dt * 128:(dt + 1) * 128, :])
            wq_sb.append(w)
        nc.sync.dma_start(cq_sb[:], cq_d[:])
        nc.sync.dma_start(sq_sb[:], sq_d[:])
        if use_bias:
            bq_sb = cst.tile([128, HPC], f32, tag="bq", name="bq_sb")
            bk_sb = cst.tile([128, HPC], f32, tag="bk", name="bk_sb")
            bv_sb = cst.tile([1, CW], f32, tag="bv", name="bv_sb")
            bo_sb = cst.tile([1, CW], f32, tag="bo", name="bo_sb")
            nc.sync.dma_start(bq_sb[:], bq_d[:])
            nc.sync.dma_start(bk_sb[:], bk_d[:])
            nc.sync.dma_start(bv_sb[:], bv_d[:])
            nc.sync.dma_start(bo_sb[:], bo_d[:])
            bvb_sb = cst.tile([128, CW], f32, tag="bvb", name="bvb_sb")
            bob_sb = cst.tile([128, CW], f32, tag="bob", name="bob_sb")
            nc.gpsimd.partition_broadcast(bvb_sb[:], bv_sb[0:1, :])
            nc.gpsimd.partition_broadcast(bob_sb[:], bo_sb[0:1, :])
        ones_sb = cst.tile([128, 128], bf16, tag="ones", name="ones_sb")
        nc.vector.memset(ones_sb[:], 1.0)
        if causal:
            tri_sb = cst.tile([128, 128], bf16, tag="tri", name="tri_sb")
            nc.sync.dma_start(tri_sb[:], dm_d[:])

        # Wo streams in behind everything else
        for dt in range(NDT):
            t = wp.tile([128, CW], bf16, tag="w", name=f"wo{dt}")
            nc.sync.dma_start(t[:], wo_d[dt * 128:(dt + 1) * 128, :])
            wo_sb.append(t)

        # persistent KT (written chunk by chunk; all history needed) and V;
        # QT is per-chunk only
        ktr = [qkp.tile([128, S], bf16, tag="qk", name=f"ktr{m}", bufs=HPC)
               for m in range(HPC)]
        v_sb = [None] * NST

        def proj_chunk_qk(w_sb, b_sb, c_sb, s_sb, dsts, dsls, hTc, c, prefix):
            """Project chunk c of Q or K into dsts[m][:, dsls[m]] + RoPE."""
            csl = slice(c * QCH, (c + 1) * QCH)  # rope-table columns
            for m in range(HPC):
                dst, dsl = dsts[m], dsls[m]
                ps = ps_mm.tile([128, QCH], f32, tag="mm",
                                name=f"{prefix}ps{m}_{c}")
                for dt in range(NDT):
                    nc.tensor.matmul(ps[:], w_sb[dt][:, m * 128:(m + 1) * 128],
                                     hTc[dt][:],
                                     start=(dt == 0), stop=(dt == NDT - 1))
                if use_bias:
                    nc.scalar.activation(dst[:, dsl], ps[:], IDN,
                                         bias=b_sb[:, m:m + 1])
                else:
                    nc.scalar.activation(dst[:, dsl], ps[:], IDN)
                # RoPE in place. rows 0:64 = "real"(a), 64:128 = "imag"(b).
                t1 = rp.tile([128, QCH], bf16, tag="t1", name=f"{prefix}t1{m}_{c}")
                t2 = rp.tile([128, QCH], bf16, tag="t2", name=f"{prefix}t2{m}_{c}")
                # t1[0:64] = b, t1[64:128] = a (swapped copy via DMA)
                nc.sync.dma_start(t1[0:64, :], dst[64:128, dsl])
                nc.sync.dma_start(t1[64:128, :], dst[0:64, dsl])
                nc.vector.tensor_mul(t2[64:128, :], dst[64:128, dsl],
                                     c_sb[64:128, csl])          # b*cos
                nc.vector.tensor_mul(dst[64:128, dsl], t1[64:128, :],
                                     s_sb[64:128, csl])          # a*sin
                nc.vector.tensor_add(dst[64:128, dsl],
                                     dst[64:128, dsl], t2[64:128, :])
                nc.vector.tensor_mul(t2[0:64, :], t1[0:64, :],
                                     s_sb[0:64, csl])            # b*sin
                nc.vector.tensor_mul(dst[0:64, dsl], dst[0:64, dsl],
                                     c_sb[0:64, csl])            # a*cos
                nc.vector.tensor_sub(dst[0:64, dsl],
                                     dst[0:64, dsl], t2[0:64, :])

        def attention_chunk(qc, qtrc):
            # Split the last chunk's AllGather into two column halves so the
            # tail AG overlaps the first half's output projection.
            if qc == NQC - 1:
                halves = [(0, QCH // 2), (QCH // 2, QCH // 2)]
            else:
                halves = [(0, QCH)]
            agins, agouts_l = [], []
            for i, (o, w) in enumerate(halves):
                agins.append(dram.tile([CW, w], bf16, tag=f"agin{qc}_{i}",
                                       name=f"agin{qc}_{i}"))
                agouts_l.append(dram.tile([D, w], bf16, tag=f"agout{qc}_{i}",
                                          name=f"agout{qc}_{i}"))
            last_mm = None
            for h in range(HPC):
                nk = 4 * qc + 4 if causal else NST
                pv = ps_pv.tile([128, QCH], f32, tag="pv", name=f"pv{h}_{qc}")
                sacc = ptp.tile([128, QCH], f32, tag="sacc",
                                name=f"sacc{h}_{qc}", bufs=2)
                for ki in range(nk):
                    p = ki - 4 * qc if causal else -1
                    c0 = max(0, 128 * p)
                    ss = ps_s.tile([128, QCH], f32, tag="s",
                                   name=f"ss{h}_{qc}_{ki}")
                    nc.tensor.matmul(
                        ss[:, c0:], ktr[h][:, ki * 128:(ki + 1) * 128],
                        qtrc[h][:, c0:], start=True, stop=True)
                    if causal:
                        if p >= 0:
                            nc.vector.tensor_add(ss[:, c0:c0 + 128],
                                                 ss[:, c0:c0 + 128], tri_sb[:])
                    else:
                        mt = ptp.tile([128, QCH], bf16, tag="mt",
                                      name=f"mt{h}_{qc}_{ki}", bufs=2)
                        nc.sync.dma_start(
                            mt[:], mT_d[ki * 128:(ki + 1) * 128,
                                        qc * QCH:(qc + 1) * QCH])
                        nc.vector.tensor_add(ss[:], ss[:], mt[:])
                    pt = ptp.tile([128, QCH], bf16, tag="pt",
                                  name=f"pt{h}_{qc}_{ki}", bufs=4)
                    if c0 > 0:
                        nc.vector.memset(pt[:, 0:c0], 0.0)
                    nc.scalar.activation(pt[:, c0:], ss[:, c0:], EXP)
                    # rowsum accumulation (f32, SBUF); partition-reduce later.
                    # Final add downcasts to bf16 so the reduce matmul is bf16.
                    if ki == 0:
                        nc.vector.tensor_copy(sacc[:], pt[:])
                    elif ki == nk - 1:
                        saccb = ptp.tile([128, QCH], bf16, tag="saccb",
                                         name=f"saccb{h}_{qc}", bufs=2)
                        nc.vector.tensor_add(saccb[:], sacc[:], pt[:])
                    else:
                        nc.vector.tensor_add(sacc[:, c0:], sacc[:, c0:],
                                             pt[:, c0:])
                    last_mm = nc.tensor.matmul(
                        pv[:], v_sb[ki][:, h * 128:(h + 1) * 128],
                        pt[:], start=(ki == 0), stop=(ki == nk - 1))
                # partition-reduce+broadcast the rowsums in one bf16 matmul
                sm = ps_s.tile([128, QCH], f32, tag="s", name=f"sm{h}_{qc}")
                nc.tensor.matmul(sm[:], ones_sb[:], saccb[:],
                                 start=True, stop=True)
                recb = ptp.tile([128, QCH], f32, tag="recb",
                                name=f"recb{h}_{qc}", bufs=2)
                nc.vector.reciprocal_approx_fast(out=recb[:], in_=sm[:])
                ot = op.tile([128, QCH], bf16, tag="ot", name=f"ot{h}_{qc}",
                             bufs=2)
                nc.vector.tensor_mul(ot[:], pv[:], recb[:])
                for i, (o, w) in enumerate(halves):
                    nc.sync.dma_start(
                        agins[i][h * 128:(h + 1) * 128, :], ot[:, o:o + w])

            pieces = []
            for i, (o, w) in enumerate(halves):
                nc.gpsimd.collective_compute(
                    "AllGather", mybir.AluOpType.bypass,
                    replica_groups=REPLICA_GROUPS,
                    ins=[agins[i][:].opt()], outs=[agouts_l[i][:].opt()])
                pieces.append((agouts_l[i], o, w))
            return pieces, last_mm

        def outproj_chunk(qc, pieces, after_mm=None):
            for pi, (agout, o, w) in enumerate(pieces):
                xt = []
                for dt in range(NDT):
                    t = xp.tile([128, w], bf16, tag="xt",
                                name=f"xt{qc}_{pi}_{dt}")
                    nc.sync.dma_start(t[:], agout[dt * 128:(dt + 1) * 128, :])
                    xt.append(t)
                for st4 in range(w // 128):
                    ps = ps_mo.tile([128, CW], f32, tag="mo",
                                    name=f"pso{qc}_{pi}_{st4}")
                    for dt in range(NDT):
                        mm = nc.tensor.matmul(
                            ps[:], xt[dt][:, st4 * 128:(st4 + 1) * 128],
                            wo_sb[dt][:], start=(dt == 0), stop=(dt == NDT - 1))
                        if dt == 0 and after_mm is not None:
                            # Keep outproj(qc) behind attention(qc+1) in PE
                            # program order: the static scheduler
                            # underestimates AllGather latency and would
                            # hoist it otherwise.
                            add_dep_helper(mm.ins, after_mm.ins, sync=False,
                                           reason="outproj after next attn")
                    row = qc * QCH + o + st4 * 128
                    of = op.tile([128, CW], f32, tag="of",
                                 name=f"of{qc}_{pi}_{st4}", bufs=2)
                    if use_bias:
                        nc.vector.tensor_add(of[:], ps[:], bob_sb[:])
                    else:
                        nc.scalar.activation(of[:], ps[:], IDN)
                    nc.sync.dma_start(out_d[row:row + 128, :], of[:])

        # ---- main pipeline over q-chunks ----
        # hT(c+1) DMAs are emitted at the top of block c (one chunk ahead) so
        # they precede attention(c)'s dependent DMAs in the queue order and
        # never sit behind an AllGather-gated transfer.
        agouts, last_mms = {}, {}
        hTcs = {0: hTc0}
        qtc_bufs = HPC + 1 if causal else 4 * HPC

        def proj_block(c):
            if c + 1 < NQC:
                nsl = slice((c + 1) * QCH, (c + 2) * QCH)
                nxt = []
                for dt in range(NDT):
                    t = hp.tile([128, QCH], bf16, tag="hT",
                                name=f"hT{c + 1}_{dt}")
                    nc.sync.dma_start(t[:], hT_d[dt * 128:(dt + 1) * 128, nsl])
                    nxt.append(t)
                hTcs[c + 1] = nxt
            hTc = hTcs[c]
            # V for s-tiles 4c..4c+3
            for sti in range(4):
                st = 4 * c + sti
                ps = ps_mm.tile([128, CW], f32, tag="mm", name=f"psv{st}")
                for dt in range(NDT):
                    nc.tensor.matmul(ps[:], hTc[dt][:, sti * 128:(sti + 1) * 128],
                                     wv_sb[dt][:],
                                     start=(dt == 0), stop=(dt == NDT - 1))
                vt = vp.tile([128, CW], bf16, tag="v", name=f"v{st}")
                if use_bias:
                    nc.vector.tensor_add(vt[:], ps[:], bvb_sb[:])
                else:
                    nc.scalar.activation(vt[:], ps[:], IDN)
                v_sb[st] = vt
            # Q first: attention's early k-tiles only need older K chunks,
            # so it can start as soon as Q(c) heads are roped.
            qtrc = [qkp.tile([128, QCH], bf16, tag="qtc", name=f"qtc{c}_{m}",
                             bufs=qtc_bufs) for m in range(HPC)]
            proj_chunk_qk(wq_sb, bq_sb if use_bias else None, cq_sb, sq_sb,
                          qtrc, [slice(0, QCH)] * HPC, hTc, c, "q")
            proj_chunk_qk(wk_sb, bk_sb if use_bias else None, ck_sb, sk_sb,
                          ktr, [slice(c * QCH, (c + 1) * QCH)] * HPC,
                          hTc, c, "k")
            return qtrc

        if causal:
            for c in range(NQC):
                qtrc = proj_block(c)
                agouts[c], last_mms[c] = attention_chunk(c, qtrc)
                if c > 0:
                    outproj_chunk(c - 1, agouts[c - 1], last_mms[c])
            outproj_chunk(NQC - 1, agouts[NQC - 1])
        else:
            # non-causal: attention(c) needs the FULL K/V, so project
            # everything first, then run the attention/AG/outproj pipeline
            qtrcs = [proj_block(c) for c in range(NQC)]
            for c in range(NQC):
                agouts[c], last_mms[c] = attention_chunk(c, qtrcs[c])
                if c > 0:
                    outproj_chunk(c - 1, agouts[c - 1], last_mms[c])
            outproj_chunk(NQC - 1, agouts[NQC - 1])

    nc.compile()
    return nc


def _get_built(causal: bool, use_bias: bool):
    key = (causal, use_bias)
    if key not in _built:
        _built[key] = _build(causal, use_bias)
    return _built[key]


def _prep_inputs(inputs, causal, use_bias):
    hs = np.asarray(inputs["hidden_states"], np.float32)
    fc = np.asarray(inputs["freqs_cis"], np.float32)
    Wq = np.asarray(inputs["Wq"], np.float32)
    Wk = np.asarray(inputs["Wk"], np.float32)
    Wv = np.asarray(inputs["Wv"], np.float32)
    Wo = np.asarray(inputs["Wo"], np.float32)
    bq = np.asarray(inputs["bq"], np.float32)
    bk = np.asarray(inputs["bk"], np.float32)
    bv = np.asarray(inputs["bv"], np.float32)
    bo = np.asarray(inputs["bo"], np.float32)

    # de-interleave permutation per 128-row head block: [0,2,..,126, 1,3,..,127]
    perm1 = np.concatenate([np.arange(0, DH, 2), np.arange(1, DH, 2)])
    permC = (np.arange(CW) // DH) * DH  # head base offsets
    perm = permC + perm1[np.arange(CW) % DH]

    scale = 1.0 / math.sqrt(DH)
    cos = np.concatenate([fc[:, :, 0].T, fc[:, :, 0].T])  # [128, S], dup halves
    sin = np.concatenate([fc[:, :, 1].T, fc[:, :, 1].T])
    cq = np.ascontiguousarray(cos * scale).astype(BF)
    sq = np.ascontiguousarray(sin * scale).astype(BF)
    ck = np.ascontiguousarray(cos).astype(BF)
    sk = np.ascontiguousarray(sin).astype(BF)

    if causal:
        tri = np.where(np.arange(128)[:, None] > np.arange(128)[None, :],
                       np.float32(NEG), np.float32(0.0)).astype(BF)
    else:
        maskT = np.ascontiguousarray(
            np.asarray(inputs["mask"], np.float32)[0, 0].T).astype(BF)

    hTb = [np.ascontiguousarray(hs[b].T).astype(BF) for b in range(B)]

    in_maps = []
    for c in range(NCORES):
        b, hg = divmod(c, GPC)
        sl = slice(CW * hg, CW * (hg + 1))
        wq_s = Wq[sl][perm]
        wk_s = Wk[sl][perm]
        m = {
            "hiddenT": hTb[b],
            "wqT": np.ascontiguousarray(wq_s.T).astype(BF),
            "wkT": np.ascontiguousarray(wk_s.T).astype(BF),
            "wvT": np.ascontiguousarray(Wv[sl].T).astype(BF),
            "woT": np.ascontiguousarray(Wo[sl].T).astype(BF),
            "cq": cq, "sq": sq, "ck": ck, "sk": sk,
        }
        if use_bias:
            m["bqp"] = np.ascontiguousarray(
                bq[sl][perm].reshape(HPC, 128).T).astype(np.float32)
            m["bkp"] = np.ascontiguousarray(
                bk[sl][perm].reshape(HPC, 128).T).astype(np.float32)
            m["bv2"] = bv[sl].reshape(1, CW).astype(np.float32)
            m["bo2"] = bo[sl].reshape(1, CW).astype(np.float32)
        if causal:
            m["dmask"] = tri
        else:
            m["maskT"] = maskT
        in_maps.append(m)
    return in_maps


def _is_causal(mask):
    mask = np.asarray(mask, np.float32)
    if mask.shape != (1, 1, S, S):
        return False
    m = mask[0, 0]
    expect = np.triu(np.full((S, S), np.float32(NEG)), k=1)
    return bool(np.array_equal(m, expect))


def run_on_cores(inputs, trace=False):
    """Compile+run; returns BassKernelResults."""
    from concourse.bass_utils import run_bass_kernel_spmd
    causal = _is_causal(inputs["mask"])
    use_bias = any(
        np.any(np.asarray(inputs[k])) for k in ("bq", "bk", "bv", "bo"))
    nc = _get_built(causal, use_bias)
    in_maps = _prep_inputs(inputs, causal, use_bias)
    r = run_bass_kernel_spmd(nc, in_maps, list(range(NCORES)), trace=trace)
    return r


def kernel(**inputs) -> np.ndarray:
    r = run_on_cores(inputs)
    out = np.empty((B, S, D), np.float32)
    for c in range(NCORES):
        b, hg = divmod(c, GPC)
        out[b, :, CW * hg:CW * (hg + 1)] = r.results[c]["out"]
    return out

